# revision 1
# baseline (speedup 1.0000x reference)
"""Trainium2 Bass kernel for nn_CNNseq_15564961481149 (dense_cnn).

Computes: embed lookup -> 3 parallel 1-D convs (K=3,4,5, channels-first)
-> bias -> max-over-time -> concat -> relu, for text [16, 64, 128] over a
[30000, 512] embedding table, F=256 filters per conv.

Strategy (pure data parallel over 8 NeuronCores):
  - Flatten B*S = 1024 samples; 128 samples per core.
  - Embedding table converted to fp16 on host, gathered on-device with
    dma_gather(transpose=True): the gathered tile lands as
    [d%128 (partition), d//128 (chunk), token] -- exactly the moving-operand
    layout the PE needs (contraction dim on partitions).
  - Conv-as-matmul: for each tap j and d-chunk c, a [128d, 128f] stationary
    tile multiplies the token columns shifted by j; accumulated in PSUM over
    all (c, j).  4 samples per matmul via a 2-level free dim (4 x L_out <= 512).
  - max-over-time on DVE straight out of PSUM, bias+relu folded in after
    (max(y + b) == max(y) + b), PE-transpose of the [f, sample] result so the
    final DMA to DRAM is fully contiguous.

Inputs are fp16-quantized for the matmul (fp32 accumulate in PSUM); measured
end-to-end max elementwise relative error vs fp64 is ~7e-4.
"""

from contextlib import ExitStack

import numpy as np

import concourse.mybir as mybir
import concourse.tile as tile
from concourse import bacc
from concourse.bass_utils import run_bass_kernel_spmd
from concourse.masks import make_identity

# Problem constants (hardcoded per harness contract).
B, S, L, D, F, V = 16, 64, 128, 512, 256, 30000
N_CORES = 8
NSAMP = B * S // N_CORES          # 128 samples per core
KS = (3, 4, 5)                    # conv kernel sizes
SLOT_BASE = (0, 3, 7)             # tap-slot offsets for conv3/4/5 (12 total)
N_SLOTS = 12
SPG = 4                           # samples per gather tile (512 idxs; >512 crashes the SWDGE transpose-gather)
NGT = NSAMP // SPG                # gather tiles per core
GPT = SPG // 4                    # groups of 4 samples per gather tile

f16 = mybir.dt.float16
f32 = mybir.dt.float32
i16 = mybir.dt.int16


def build_nc(nsamp=NSAMP, spg=SPG, mode="full"):
    """Build the per-core Bass program (SPMD: same program, 8 cores).

    mode: "full" | "nogather" (memset x tiles) | "nomm" (skip matmul/reduce)
    -- the reduced modes exist only to attribute wall-clock time.
    """
    ngt = nsamp // spg
    gpt = spg // 4
    t_tot = nsamp * L                  # tokens per core
    n_idx = spg * L                    # tokens per gather
    idx_cols_per_gather = n_idx // 16

    nc = bacc.Bacc("TRN2", target_bir_lowering=False, debug=False,
                   num_devices=N_CORES)

    emb_h = nc.dram_tensor("emb", [V, D], f16, kind="ExternalInput")
    idx_h = nc.dram_tensor("idx", [128, t_tot // 16], i16, kind="ExternalInput")
    wst_h = nc.dram_tensor("wst", [128, N_SLOTS, 4, 2, 128], f16,
                           kind="ExternalInput")
    bias_h = nc.dram_tensor("bias", [128, 6], f32, kind="ExternalInput")
    out_h = nc.dram_tensor("out", [nsamp, 3 * F], f32, kind="ExternalOutput")

    with tile.TileContext(nc) as tc, ExitStack() as ctx:
        cpool = ctx.enter_context(tc.tile_pool(name="consts", bufs=1))
        xpool = ctx.enter_context(tc.tile_pool(name="x", bufs=6))
        pspool = ctx.enter_context(
            tc.tile_pool(name="ps", bufs=6, space="PSUM"))
        tppool = ctx.enter_context(
            tc.tile_pool(name="tp", bufs=2, space="PSUM"))

        idx_sb = cpool.tile([128, t_tot // 16], i16)
        w_sb = cpool.tile([128, N_SLOTS, 4, 2, 128], f16)
        bias_sb = cpool.tile([128, 6], f32)
        ident = cpool.tile([128, 128], f32)
        out_sb = cpool.tile([128, 6, nsamp], f32)
        out_t = cpool.tile([nsamp, 6 * 128], f32)

        nc.sync.dma_start(out=idx_sb[:], in_=idx_h.ap()[:])
        nc.sync.dma_start(out=w_sb[:], in_=wst_h.ap()[:])
        nc.sync.dma_start(out=bias_sb[:], in_=bias_h.ap()[:])
        make_identity(nc, ident[:])

        if mode == "nomm":
            nc.gpsimd.memset(out_sb[:], 0.0)
        reps = int(mode[len("repeat"):]) if mode.startswith("repeat") else 0
        loop_cm = tc.For_i(0, reps, 1) if reps else None
        if loop_cm is not None:
            loop_cm.__enter__()
        # Process gather tiles in batches of QB so each stationary weight tile
        # is reused across QB matmuls (amortizes LDWEIGHTS 4x).
        # QB>1 (stationary reuse across gather tiles) measured ~28% SLOWER on
        # HW than back-to-back same-bank matmuls: LDWEIGHTS is already hidden
        # by the PE reorder window + dual SBUF read ports, and interleaving
        # PSUM banks/operand buffers per instruction costs more than it saves.
        QB = 1
        for sup in range(ngt // QB):
            xvs = []
            for q in range(QB):
                t = sup * QB + q
                xt = xpool.tile([128, 4, n_idx], f16, tag="xt")
                if mode == "nogather":
                    nc.gpsimd.memset(xt[:], 0.0)
                else:
                    nc.gpsimd.dma_gather(
                        out_ap=xt[:],
                        in_ap=emb_h.ap()[:],
                        idxs_ap=idx_sb[:, t * idx_cols_per_gather:
                                       (t + 1) * idx_cols_per_gather],
                        num_idxs=n_idx,
                        num_idxs_reg=n_idx,
                        elem_size=D,
                        transpose=True,
                    )
                if mode == "nomm":
                    nc.vector.tensor_copy(out_sb[:, 0, t:t + 1], xt[:, 0, :1])
                xvs.append(xt.rearrange("p c (s l) -> p c s l", s=spg))
            if mode == "nomm":
                continue
            for k_idx, K in enumerate(KS):
                lout = L - K + 1
                for m in range(2):
                    pss = [pspool.tile([128, 4, lout], f32, tag="ps",
                                       name=f"ps_{sup}_{k_idx}_{m}_{q}")
                           for q in range(QB)]
                    n_mm = 4 * K
                    mm = 0
                    for c in range(4):
                        for j in range(K):
                            for q in range(QB):
                                nc.tensor.matmul(
                                    pss[q][:],
                                    w_sb[:, SLOT_BASE[k_idx] + j, c, m, :],
                                    xvs[q][:, c, 0:4, j:j + lout],
                                    start=(mm == 0),
                                    stop=(mm == n_mm - 1),
                                )
                            mm += 1
                    tile6 = k_idx * 2 + m
                    for q in range(QB):
                        gidx = sup * QB + q
                        nc.vector.reduce_max(
                            out_sb[:, tile6, gidx * 4:gidx * 4 + 4],
                            pss[q][:],
                            axis=mybir.AxisListType.X,
                        )

        if loop_cm is not None:
            loop_cm.__exit__(None, None, None)
        # bias + relu on [f(partition), sample] layout, then PE-transpose so
        # the final DMA writes contiguous [sample, 768] rows.
        for tile6 in range(6):
            nc.vector.tensor_scalar(
                out_sb[:, tile6, :], out_sb[:, tile6, :],
                bias_sb[:, tile6:tile6 + 1], 0.0,
                op0=mybir.AluOpType.add, op1=mybir.AluOpType.max,
            )
            tp = tppool.tile([nsamp, 128], f32, tag="tp")
            nc.tensor.transpose(tp[:], out_sb[:, tile6, :], ident[:])
            nc.vector.tensor_copy(
                out_t[:, tile6 * 128:(tile6 + 1) * 128], tp[:])
        nc.sync.dma_start(out=out_h.ap()[:], in_=out_t[:])

    nc.compile()
    return nc


def prep_inputs(text, embed, w3, b3, w4, b4, w5, b5, nsamp=NSAMP, spg=SPG,
                n_cores=N_CORES):
    """Host-side marshaling: shard text, wrap gather indices, fp16-quantize
    and retile the weights/embedding."""
    text = np.ascontiguousarray(np.asarray(text).reshape(B * S, L))
    assert text.max() < V and text.min() >= 0
    emb16 = np.ascontiguousarray(np.asarray(embed, dtype=np.float16))

    wst = np.zeros((128, N_SLOTS, 4, 2, 128), np.float16)
    for k_idx, w in enumerate((w3, w4, w5)):
        w = np.asarray(w, dtype=np.float32)
        for j in range(KS[k_idx]):
            # wst[dd, slot, c, m, ff] = w[m*128+ff, c*128+dd, j]
            wj = w[:, :, j].reshape(2, 128, 4, 128)      # [m, ff, c, dd]
            wst[:, SLOT_BASE[k_idx] + j] = wj.transpose(3, 2, 0, 1)
    wst = np.ascontiguousarray(wst)

    bias = np.zeros((128, 6), np.float32)
    for k_idx, b in enumerate((b3, b4, b5)):
        bias[:, 2 * k_idx:2 * k_idx + 2] = \
            np.asarray(b, dtype=np.float32).reshape(2, 128).T
    bias = np.ascontiguousarray(bias)

    ngt = nsamp // spg
    in_maps = []
    for r in range(n_cores):
        tcore = text[r * nsamp:(r + 1) * nsamp].astype(np.int16)
        # token i of gather tile t -> partition i%16, column t*(spg*L/16)+i//16;
        # the 16-row block must be replicated to all 128 partitions (each of
        # the 8 gpsimd sub-cores reads its own 16-partition stripe).
        a = tcore.reshape(ngt, spg * L // 16, 16)         # [t, c, p]
        idx = np.tile(a.transpose(2, 0, 1).reshape(16, -1), (8, 1))
        in_maps.append({
            "emb": emb16,
            "idx": np.ascontiguousarray(idx),
            "wst": wst,
            "bias": bias,
        })
    return in_maps


_CACHE = {}


def kernel(text, embed, w3, b3, w4, b4, w5, b5):
    if "nc" not in _CACHE:
        _CACHE["nc"] = build_nc()
    nc = _CACHE["nc"]
    in_maps = prep_inputs(text, embed, w3, b3, w4, b4, w5, b5)
    res = run_bass_kernel_spmd(nc, in_maps, list(range(N_CORES)))
    out = np.concatenate([res.results[r]["out"] for r in range(N_CORES)],
                         axis=0)
    return out.reshape(B, S, 3 * F).astype(np.float32)



# revision 2
# speedup vs baseline: 1.2311x; 1.2311x over previous
"""Trainium2 Bass kernel for nn_CNNseq_15564961481149 (dense_cnn).

Computes: embed lookup -> 3 parallel 1-D convs (K=3,4,5, channels-first)
-> bias -> max-over-time -> concat -> relu, for text [16, 64, 128] over a
[30000, 512] embedding table, F=256 filters per conv.

Strategy (pure data parallel over 8 NeuronCores):
  - Flatten B*S = 1024 samples; 128 samples per core.
  - Embedding table converted to fp16 on host, gathered on-device with
    dma_gather(transpose=True): the gathered tile lands as
    [d%128 (partition), d//128 (chunk), token] -- exactly the moving-operand
    layout the PE needs (contraction dim on partitions).
  - Conv-as-matmul: for each tap j and d-chunk c, a [128d, 128f] stationary
    tile multiplies the token columns shifted by j; accumulated in PSUM over
    all (c, j).  4 samples per matmul via a 2-level free dim (4 x L_out <= 512
    fp32 = one PSUM bank; a matmul output cannot cross banks).
  - Per conv K one 2-bank PSUM tile holds both f-halves; matmuls stay
    same-bank back-to-back within each half (bank alternation between
    consecutive matmuls measured 2x slower).
  - PSUM evacuation on the otherwise-idle ACT engine (Copy f32->fp16 into
    SBUF; ScalarE has its own PSUM port), then max-over-time on DVE from
    SBUF in fp16 (2x mode) -- keeps DVE off the single PSUM read port that
    otherwise contends with the PE.
  - Deep gather prefetch (10 x-buffers) hides SWDGE latency jitter; shallow
    (6) measured ~125us slower.
  - Epilogue: relu(x+bias) on ACT, PE-transpose of [f, sample] to sample-major
    fp16, cast to f32 on the final copy so the output DMA is contiguous f32.

fp16 quantization of embeddings/weights (fp32 PSUM accumulate) plus the fp16
max pipeline gives max elementwise relative error ~1.4e-3 vs the fp32
reference (gate is 2e-2).
"""

from contextlib import ExitStack

import numpy as np

import concourse.mybir as mybir
import concourse.tile as tile
from concourse import bacc
from concourse.bass_utils import run_bass_kernel_spmd
from concourse.masks import make_identity

# Problem constants (hardcoded per harness contract).
B, S, L, D, F, V = 16, 64, 128, 512, 256, 30000
N_CORES = 8
NSAMP = B * S // N_CORES          # 128 samples per core
KS = (3, 4, 5)                    # conv kernel sizes
SLOT_BASE = (0, 3, 7)             # tap-slot offsets for conv3/4/5 (12 total)
N_SLOTS = 12
SPG = 4                           # samples per gather (512 idxs; >512 crashes
                                  # the SWDGE transpose-gather)
NGT = NSAMP // SPG                # 32 gather tiles per core
XBUFS = 10                        # gather prefetch depth

f16 = mybir.dt.float16
f32 = mybir.dt.float32
i16 = mybir.dt.int16


def build_nc(nsamp=NSAMP, spg=SPG, mode="full"):
    """Build the per-core Bass program (SPMD: same program, 8 cores).

    mode: "full" | "nogather" | "nomm" | "repeatN" -- reduced modes exist
    only to attribute wall-clock time in test.py.
    """
    ngt = nsamp // spg
    t_tot = nsamp * L
    n_idx = spg * L
    idx_cols_per_gather = n_idx // 16

    nc = bacc.Bacc("TRN2", target_bir_lowering=False, debug=False,
                   num_devices=N_CORES)

    emb_h = nc.dram_tensor("emb", [V, D], f16, kind="ExternalInput")
    idx_h = nc.dram_tensor("idx", [128, t_tot // 16], i16, kind="ExternalInput")
    wst_h = nc.dram_tensor("wst", [128, N_SLOTS, 4, 2, 128], f16,
                           kind="ExternalInput")
    bias_h = nc.dram_tensor("bias", [128, 6], f32, kind="ExternalInput")
    out_h = nc.dram_tensor("out", [nsamp, 3 * F], f32, kind="ExternalOutput")

    with tile.TileContext(nc) as tc, ExitStack() as ctx:
        cpool = ctx.enter_context(tc.tile_pool(name="consts", bufs=1))
        xpool = ctx.enter_context(tc.tile_pool(name="x", bufs=XBUFS))
        evpool = ctx.enter_context(tc.tile_pool(name="ev", bufs=4))
        pspool = ctx.enter_context(
            tc.tile_pool(name="ps", bufs=3, space="PSUM"))
        tppool = ctx.enter_context(
            tc.tile_pool(name="tp", bufs=2, space="PSUM"))

        idx_sb = cpool.tile([128, t_tot // 16], i16)
        w_sb = cpool.tile([128, N_SLOTS, 4, 2, 128], f16)
        bias_sb = cpool.tile([128, 6], f32)
        ident = cpool.tile([128, 128], f16)
        out_sb = cpool.tile([128, 6, nsamp], f16)
        out_t = cpool.tile([nsamp, 6 * 128], f32)

        nc.sync.dma_start(out=idx_sb[:], in_=idx_h.ap()[:])
        nc.sync.dma_start(out=w_sb[:], in_=wst_h.ap()[:])
        nc.sync.dma_start(out=bias_sb[:], in_=bias_h.ap()[:])
        make_identity(nc, ident[:])

        static_x = None
        if mode == "nogather":
            static_x = [cpool.tile([128, 4, n_idx], f16, name=f"sx{i}")
                        for i in range(6)]
            for t in static_x:
                nc.gpsimd.memset(t[:], 0.0)
        if mode == "nomm":
            nc.gpsimd.memset(out_sb[:], 0.0)

        reps = int(mode[len("repeat"):]) if mode.startswith("repeat") else 0
        loop_cm = tc.For_i(0, reps, 1) if reps else None
        if loop_cm is not None:
            loop_cm.__enter__()

        for t in range(ngt):
            if mode == "nogather":
                xt = static_x[t % len(static_x)]
            else:
                xt = xpool.tile([128, 4, n_idx], f16, tag="xt",
                                name=f"xt_{t}")
                nc.gpsimd.dma_gather(
                    out_ap=xt[:],
                    in_ap=emb_h.ap()[:],
                    idxs_ap=idx_sb[:, t * idx_cols_per_gather:
                                   (t + 1) * idx_cols_per_gather],
                    num_idxs=n_idx,
                    num_idxs_reg=n_idx,
                    elem_size=D,
                    transpose=True,
                )
            xv = xt.rearrange("p c (s l) -> p c s l", s=spg)
            if mode == "nomm":
                nc.vector.tensor_copy(out_sb[:, 0, t:t + 1], xt[:, 0, :1])
                continue
            for k_idx, K in enumerate(KS):
                lout = L - K + 1
                psk = pspool.tile([128, 2, spg, 128], f32, tag="ps",
                                  name=f"psk_{t}_{k_idx}")
                for m in range(2):
                    n_mm = 4 * K
                    mmi = 0
                    for c in range(4):
                        for j in range(K):
                            nc.tensor.matmul(
                                psk[:, m, :, 0:lout],
                                w_sb[:, SLOT_BASE[k_idx] + j, c, m, :],
                                xv[:, c, :, j:j + lout],
                                start=(mmi == 0),
                                stop=(mmi == n_mm - 1),
                            )
                            mmi += 1
                ev = evpool.tile([128, 2, spg, 128], f16, tag="ev",
                                 name=f"ev_{t}_{k_idx}")
                nc.scalar.activation(
                    ev[:, :, :, 0:lout], psk[:, :, :, 0:lout],
                    func=mybir.ActivationFunctionType.Copy,
                )
                nc.vector.reduce_max(
                    out_sb[:, k_idx * 2:k_idx * 2 + 2,
                           t * spg:(t + 1) * spg],
                    ev[:, :, :, 0:lout],
                    axis=mybir.AxisListType.X,
                )

        if loop_cm is not None:
            loop_cm.__exit__(None, None, None)

        # relu(max + bias) on ACT (fp16 data, f32 scalar bias), PE-transpose
        # so the final DMA writes contiguous [sample, 768] f32 rows.
        for tile6 in range(6):
            nc.scalar.activation(
                out_sb[:, tile6, :], out_sb[:, tile6, :],
                func=mybir.ActivationFunctionType.Relu,
                bias=bias_sb[:, tile6:tile6 + 1],
            )
            tp = tppool.tile([nsamp, 128], f16, tag="tp")
            nc.tensor.transpose(tp[:], out_sb[:, tile6, :], ident[:])
            nc.vector.tensor_copy(
                out_t[:, tile6 * 128:(tile6 + 1) * 128], tp[:])
        nc.sync.dma_start(out=out_h.ap()[:], in_=out_t[:])

    nc.compile()
    return nc


def prep_inputs(text, embed, w3, b3, w4, b4, w5, b5, nsamp=NSAMP, spg=SPG,
                n_cores=N_CORES):
    """Host-side marshaling: shard text, wrap gather indices, fp16-quantize
    and retile the weights/embedding."""
    text = np.ascontiguousarray(np.asarray(text).reshape(B * S, L))
    assert text.max() < V and text.min() >= 0
    emb16 = np.ascontiguousarray(np.asarray(embed, dtype=np.float16))

    wst = np.zeros((128, N_SLOTS, 4, 2, 128), np.float16)
    for k_idx, w in enumerate((w3, w4, w5)):
        w = np.asarray(w, dtype=np.float32)
        for j in range(KS[k_idx]):
            # wst[dd, slot, c, m, ff] = w[m*128+ff, c*128+dd, j]
            wj = w[:, :, j].reshape(2, 128, 4, 128)      # [m, ff, c, dd]
            wst[:, SLOT_BASE[k_idx] + j] = wj.transpose(3, 2, 0, 1)
    wst = np.ascontiguousarray(wst)

    bias = np.zeros((128, 6), np.float32)
    for k_idx, b in enumerate((b3, b4, b5)):
        bias[:, 2 * k_idx:2 * k_idx + 2] = \
            np.asarray(b, dtype=np.float32).reshape(2, 128).T
    bias = np.ascontiguousarray(bias)

    ngt = nsamp // spg
    in_maps = []
    for r in range(n_cores):
        tcore = text[r * nsamp:(r + 1) * nsamp].astype(np.int16)
        # token i of gather tile t -> partition i%16, column t*(spg*L/16)+i//16;
        # the 16-row block must be replicated to all 128 partitions (each of
        # the 8 gpsimd sub-cores reads its own 16-partition stripe).
        a = tcore.reshape(ngt, spg * L // 16, 16)         # [t, c, p]
        idx = np.tile(a.transpose(2, 0, 1).reshape(16, -1), (8, 1))
        in_maps.append({
            "emb": emb16,
            "idx": np.ascontiguousarray(idx),
            "wst": wst,
            "bias": bias,
        })
    return in_maps


_CACHE = {}


def kernel(text, embed, w3, b3, w4, b4, w5, b5):
    if "nc" not in _CACHE:
        _CACHE["nc"] = build_nc()
    nc = _CACHE["nc"]
    in_maps = prep_inputs(text, embed, w3, b3, w4, b4, w5, b5)
    res = run_bass_kernel_spmd(nc, in_maps, list(range(N_CORES)))
    out = np.concatenate([res.results[r]["out"] for r in range(N_CORES)],
                         axis=0)
    return out.reshape(B, S, 3 * F).astype(np.float32)
